# revision 26
# baseline (speedup 1.0000x reference)
"""TRN2 Bass kernel for nn_MultiBlockStructuredScoreNet.

Computes s(z) = -grad_z U(z) where
  U(z) = sum_k MLP_k(z_k) + sum_r z_8^T W_r z_{8-r}
for z of shape (8192, 9*256), data-parallel over 8 NeuronCores.

Per core (1024 samples):
 - Host pre-transposes z to neuron-major (zT) so the PE contracts over the
   neuron dim with no on-chip transposes; host packs/transposes/sign-flips
   the small parameters so PSUM accumulation directly yields the score.
 - fp16 PE dtype and fp16 stores (cast back to f32 on host): rel err
   ~5e-4 against the 2e-2 gate, and the store traffic halves vs f32.
 - u1 lhsT tiles are densely packed (32 cols each) and the params DMA is
   split so the HAM warm-up matmuls only wait on a 147KB slice.
 - Emission order bridges DMA waits with real work: tile-1's MLP forward
   runs while the coupling matrices stream, so the PE never idles on the
   cparams DMA (the old kernel burned throwaway matmuls there).
 - MLP: u1 via col-tiled [K=128,M=32] matmuls packing 4 blocks per PSUM
   tile; u2/dh1 via block-diagonal [128,128] weights with -gW3 pre-folded
   into the dh1 weights; SiLU / SiLU' from the ACT table with fused bias,
   batched by function to avoid table reloads.
 - Cross couplings + MLP d_z accumulate b-major into PSUM per 128-sample
   chunk; adjacent blocks are paired into single N=512 matmuls (d_lag via
   adjacent wlag packing, d_z via block-diagonal W1^T pairs) to halve the
   weight-load count.  PSUM->SBUF copies split across DVE/ACT; stores are
   fp16 and the final chunk stores in 3 pieces to shorten the tail.
"""

import numpy as np
import ml_dtypes

import concourse.bass as bass
import concourse.tile as tile
from concourse import bacc, mybir
from concourse.bass_utils import run_bass_kernel_spmd

AF = mybir.ActivationFunctionType
F32 = mybir.dt.float32

N_CORES = 8
BATCH = 8192
B_CORE = BATCH // N_CORES     # 1024
BT = 512                      # batch tile (PSUM free-dim max for f32)
NBT = B_CORE // BT            # 2 batch tiles per core
NCHUNK = 4                    # 128-sample chunks per batch tile
NB = 9                        # blocks
P_MAX = 8
NN = 256                      # neurons per block
D = NB * NN                   # 2304
H = 32

ZW = 2 * NB * BT              # zt tile cols per batch tile: 18 chunks x 512
OFF_U1 = 0                    # dense u1 lhsT tiles: 18 x 32 cols
OFF_W1T = 18 * 32             # 576: paired block-diag W1^T region
OFF_W2BD = OFF_W1T + 4 * 512 + 256   # 2880
OFF_W2TBD = OFF_W2BD + 3 * 128       # 3264
PW = OFF_W2TBD + 3 * 128             # 3648
PA_SPLIT = OFF_W1T            # params DMA A/B split: warmup+u1 vs the rest
OFF_WFUT = 2 * P_MAX * NN     # 4096
CW = 2 * OFF_WFUT             # 8192

MM_MODE = "fp16"              # "fp16" | "bf16"

_DT = {
    "fp16": (mybir.dt.float16, np.float16),
    "bf16": (mybir.dt.bfloat16, ml_dtypes.bfloat16),
}


def _body(tc, out, zt, params, cparams, biases, ctx):
    nc = tc.nc
    sdt = _DT[MM_MODE][0]

    const = ctx.enter_context(tc.tile_pool(name="const", bufs=1))
    ztp = ctx.enter_context(tc.tile_pool(name="ztp", bufs=2))
    mlpp = ctx.enter_context(tc.tile_pool(name="mlpp", bufs=3, space="PSUM"))
    actp = ctx.enter_context(tc.tile_pool(name="actp", bufs=8))
    du1p = ctx.enter_context(tc.tile_pool(name="du1p", bufs=6))
    outp = ctx.enter_context(tc.tile_pool(name="outp", bufs=5, space="PSUM"))
    outs = ctx.enter_context(tc.tile_pool(name="outs", bufs=4))

    # ---- inputs -> SBUF in fine-grained pieces ordered by PE consumption:
    # u1 weights (slice A) + the first z chunks arrive ~7.5us in so the real
    # u1 matmuls double as the HAM ramp (no throwaway warm-ups); the HAM
    # clamps the PE hard if the ramp is followed by idle-then-burst, so the
    # pieces trickle in to keep the PE continuously fed from first matmul
    # to last.  cparams lands before zt1: the tile-0 cross chunks only need
    # cparams + zt0 (their dz matmuls come last, after the W1^T slice B2).
    pa_sb = const.tile([128, PW], sdt, name="pa")
    nc.sync.dma_start(pa_sb[:, 0:PA_SPLIT], params[:, 0:PA_SPLIT])
    bias_sb = const.tile([128, 6], F32, name="biassb")
    nc.sync.dma_start(bias_sb[:], biases[:])
    zt_sb = [ztp.tile([128, ZW], sdt, tag="zt", name="ztsb") for _ in range(NBT)]
    cp_sb = const.tile([128, CW], sdt, name="cp")
    nc.sync.dma_start(zt_sb[0][:, 16 * BT:ZW], zt[0, :, 16 * BT:ZW])
    for q in range(4):
        nc.sync.dma_start(zt_sb[0][:, 4 * BT * q:4 * BT * (q + 1)],
                          zt[0, :, 4 * BT * q:4 * BT * (q + 1)])
    nc.sync.dma_start(pa_sb[:, OFF_W2BD:PW], params[:, OFF_W2BD:PW])   # B1
    nc.sync.dma_start(zt_sb[1][:, 16 * BT:ZW], zt[1, :, 16 * BT:ZW])
    for q in range(4):
        nc.sync.dma_start(zt_sb[1][:, 4 * BT * q:4 * BT * (q + 1)],
                          zt[1, :, 4 * BT * q:4 * BT * (q + 1)])
    for q in range(4):
        nc.sync.dma_start(cp_sb[:, 2048 * q:2048 * (q + 1)],
                          cparams[:, 2048 * q:2048 * (q + 1)])
    nc.sync.dma_start(pa_sb[:, PA_SPLIT:OFF_W2BD], params[:, PA_SPLIT:OFF_W2BD])  # B2

    def ztsl(t, c, c0, w):    # [128, w] slice at offset c0 of zT chunk c
        return zt_sb[t][:, BT * c + c0:BT * c + c0 + w]

    u1_state = {}
    act_state = {}
    du1_tiles = {}

    def u1_group(t, g):
        nblk = 4 if g < 2 else 1
        P = 32 * nblk
        u1 = mlpp.tile([128, BT], F32, tag="mlpp", name="u1t")
        for j in range(nblk):
            k = 4 * g + j
            for hf in range(2):
                c = 2 * k + hf
                nc.tensor.matmul(
                    u1[32 * j:32 * j + 32, :],
                    pa_sb[:, 32 * c:32 * c + 32],
                    ztsl(t, c, 0, BT), start=(hf == 0), stop=(hf == 1),
                    tile_position=(0, 32 * j))
        return u1, P

    def u1_phase(t, order=(0, 1, 2)):
        u1s = [None] * 3
        for g in order:
            u1s[g] = u1_group(t, g)
        u1_state[t] = u1s

    def mlp_act_phase(t):
        # SiLU + SiLU' on ACT only -- emitted early so the ACT engine can
        # run these under PE work.
        u1s = u1_state[t]
        h1s, sp1s = [], []
        for g in range(3):
            u1, P = u1s[g]
            h1 = actp.tile([128, BT], sdt, tag="act", name="h1t")
            nc.scalar.activation(h1[:P], u1[:P], AF.Silu, bias=bias_sb[:P, g:g + 1])
            h1s.append(h1)
        for g in range(3):
            u1, P = u1s[g]
            sp1 = actp.tile([128, BT], F32, tag="act", name="sp1t")
            nc.scalar.activation(sp1[:P], u1[:P], AF.Derivative_silu,
                                 bias=bias_sb[:P, g:g + 1])
            sp1s.append(sp1)
        act_state[t] = (h1s, sp1s)

    bwd_state = {}

    def mlp_u2_phase(t):
        # u2 matmuls + sp2: first half of the backward chain
        h1s, sp1s = act_state[t]
        u1s = u1_state[t]
        u2s, sp2s = [], []
        for g in range(3):
            P = u1s[g][1]
            u2 = mlpp.tile([128, BT], F32, tag="mlpp", name="u2t")
            nc.tensor.matmul(u2[:P], pa_sb[:P, OFF_W2BD + 128 * g:OFF_W2BD + 128 * g + P],
                             h1s[g][:P], start=True, stop=True)
            u2s.append(u2)
        for g in range(3):
            P = u1s[g][1]
            sp2 = actp.tile([128, BT], sdt, tag="act", name="sp2t")
            nc.scalar.activation(sp2[:P], u2s[g][:P], AF.Derivative_silu,
                                 bias=bias_sb[:P, 3 + g:4 + g])
            sp2s.append(sp2)
        bwd_state[t] = sp2s

    def mlp_bwd_phase(t):
        # dh1 matmuls + du1: second half of the backward chain
        h1s, sp1s = act_state[t]
        u1s = u1_state[t]
        sp2s = bwd_state[t]
        du1_sb = []
        for g in range(3):
            P = u1s[g][1]
            dh1 = mlpp.tile([128, BT], F32, tag="mlpp", name="dh1t")
            # -gW3 is folded into these weights: dh1 here is -d(e)/d(h1)
            nc.tensor.matmul(dh1[:P],
                             pa_sb[:P, OFF_W2TBD + 128 * g:OFF_W2TBD + 128 * g + P],
                             sp2s[g][:P], start=True, stop=True)
            du1 = du1p.tile([128, BT], sdt, tag="du1", name="du1t")
            nc.vector.tensor_mul(du1[:P], dh1[:P], sp1s[g][:P])
            du1_sb.append(du1)
        du1_tiles[t] = du1_sb

    def cross_chunk(t, c, dve_only=False, pipelined=False, last=False,
                    tiles=None):
        du1_sb = du1_tiles[t]
        bs = slice(128 * c, 128 * c + 128)
        ot = outs.tile([128, D], sdt, tag="outs", name="outst")
        if tiles is None:
            op = [outp.tile([128, 512], F32, tag="outp", name="outpt")
                  for _ in range(4)]
            o8 = outp.tile([128, 512], F32, tag="outp", name="outpt")
        else:
            op, o8 = tiles

        def dlag(p):
            for ih in range(2):
                nc.tensor.matmul(
                    op[p][:], ztsl(t, 2 * P_MAX + ih, 128 * c, 128),
                    cp_sb[:, 2048 * ih + 512 * p:2048 * ih + 512 * p + 512],
                    start=(ih == 0), stop=False)

        def dz(p):
            base = 64 * (p % 2)
            nc.tensor.matmul(
                op[p][:], du1_sb[p // 2][base:base + 64, bs],
                pa_sb[base:base + 64, OFF_W1T + 512 * p:OFF_W1T + 512 * p + 512],
                start=False, stop=True, tile_position=(base, 0))

        def copy(p, eng):
            if eng == 'v':
                nc.vector.tensor_copy(ot[:, 512 * p:512 * (p + 1)], op[p][:])
            else:
                nc.scalar.activation(ot[:, 512 * p:512 * (p + 1)], op[p][:], AF.Copy)

        def dfut():
            for r in range(1, P_MAX + 1):
                for jh in range(2):
                    i = 2 * (r - 1) + jh
                    nc.tensor.matmul(
                        o8[:, :NN], ztsl(t, 2 * (P_MAX - r) + jh, 128 * c, 128),
                        cp_sb[:, OFF_WFUT + 256 * i:OFF_WFUT + 256 * i + 256],
                        start=(i == 0), stop=False)
            nc.tensor.matmul(o8[:, :NN], du1_sb[2][0:32, bs],
                             pa_sb[0:32, OFF_W1T + 2048:OFF_W1T + 2048 + 256],
                             start=False, stop=True, tile_position=(0, 0))

        r0 = t * BT + c * 128
        if pipelined:
            # dfut first so the tiny [2048:2304] store leaves early; the
            # last chunk splits its p3 bank into 256-col halves so the
            # final compute->copy->store tail is only ~0.6us.
            dfut()
            nc.vector.tensor_copy(ot[:, 2048:2304], o8[:, :NN])
            nc.sync.dma_start(out[r0:r0 + 128, 2048:2304], ot[:, 2048:2304])
            for p in range(4):
                dlag(p)
                dz(p)
                if last and p == 3:
                    # split the final copy across ACT+DVE so it drains fast
                    nc.scalar.activation(ot[:, 1536:1792], op[3][:, 0:256],
                                         AF.Copy)
                    nc.vector.tensor_copy(ot[:, 1792:2048], op[3][:, 256:512])
                else:
                    copy(p, 'v' if p % 2 == 0 else 's')
                if p == 1:
                    nc.sync.dma_start(out[r0:r0 + 128, 0:1024], ot[:, 0:1024])
                if last and p == 2:
                    nc.sync.dma_start(out[r0:r0 + 128, 1024:1536],
                                      ot[:, 1024:1536])
            if last:
                nc.sync.dma_start(out[r0:r0 + 128, 1536:2048], ot[:, 1536:2048])
            else:
                nc.sync.dma_start(out[r0:r0 + 128, 1024:2048], ot[:, 1024:2048])
            return
        else:
            for p in range(4):
                dlag(p)
            dfut()
            for p in range(4):
                dz(p)
            for p in range(3):
                copy(p, 'v')
            if dve_only:
                copy(3, 'v')
                nc.vector.tensor_copy(ot[:, 2048:2304], o8[:, :NN])
            else:
                copy(3, 's')
                nc.scalar.activation(ot[:, 2048:2304], o8[:, :NN], AF.Copy)
        nc.sync.dma_start(out[r0:r0 + 128, :], ot[:])

    # ---- emission order == per-engine execution order.  Tile-0 MLP ramps
    # the HAM, tile-0 cross chunks run as cparams trickles in, tile-1 MLP
    # slots in once zt1 lands, then the tile-1 cross chunks.  Filler
    # matmuls (into chunk (0,0)'s pre-allocated PSUM banks) plug the
    # data-wait gaps so the PE stays continuously active: the HAM grants
    # full rate only after ~3us of unbroken activity, and punishes
    # idle-then-burst with a multi-us 50% clamp.
    op0 = [outp.tile([128, 512], F32, tag="outp", name="outpt")
           for _ in range(4)]
    o80 = outp.tile([128, 512], F32, tag="outp", name="outpt")

    def filler(n):
        for _ in range(n):
            nc.tensor.matmul(op0[0][:], pa_sb[:, 0:128], pa_sb[:, 0:BT],
                             start=True, stop=True)

    u1s0 = [None] * 3
    u1s0[2] = u1_group(0, 2)         # g2 first: its zT chunks arrive first
    filler(3)
    u1s0[0] = u1_group(0, 0)
    filler(4)
    u1s0[1] = u1_group(0, 1)
    u1_state[0] = u1s0
    mlp_act_phase(0)
    mlp_u2_phase(0)
    mlp_bwd_phase(0)
    u1_phase(1, order=(2, 0, 1))     # zt1 trickles in during this
    mlp_act_phase(1)
    filler(4)
    cross_chunk(0, 0, tiles=(op0, o80))
    cross_chunk(0, 1)
    mlp_u2_phase(1)                  # PE bits hide between cross chunks;
    cross_chunk(0, 2)                # the ACT round-trips run off-path
    mlp_bwd_phase(1)
    cross_chunk(0, 3)
    cross_chunk(1, 0, pipelined=True)
    cross_chunk(1, 1, pipelined=True)
    cross_chunk(1, 2, pipelined=True)
    cross_chunk(1, 3, pipelined=True, last=True)


# ------------------------------------------------------------- build + launch

_CACHED = {}


def _build():
    if MM_MODE in _CACHED:
        return _CACHED[MM_MODE]
    sdt = _DT[MM_MODE][0]
    nc = bacc.Bacc("TRN2", target_bir_lowering=False, debug=False,
                   num_devices=N_CORES)
    zt = nc.dram_tensor("zt", [NBT, 128, ZW], sdt, kind="ExternalInput").ap()
    params = nc.dram_tensor("params", [128, PW], sdt, kind="ExternalInput").ap()
    cparams = nc.dram_tensor("cparams", [128, CW], sdt, kind="ExternalInput").ap()
    biases = nc.dram_tensor("biases", [128, 6], F32, kind="ExternalInput").ap()
    out = nc.dram_tensor("out", [B_CORE, D], sdt, kind="ExternalOutput").ap()

    from contextlib import ExitStack
    with tile.TileContext(nc) as tc:
        with ExitStack() as ctx:
            _body(tc, out, zt, params, cparams, biases, ctx)
    nc.compile()
    _CACHED[MM_MODE] = nc
    return nc


def _prep_params(gW1, gb1, gW2, gb2, gW3, gb3, W):
    mdt = _DT[MM_MODE][1]
    params = np.zeros((128, PW), np.float32)
    biases = np.zeros((128, 6), np.float32)
    for k in range(NB):
        g, j = k // 4, k % 4
        rs = slice(32 * j, 32 * j + 32)
        for hf in range(2):
            # u1 lhsT tile (2k+hf), densely packed at 32 cols each
            c = 2 * k + hf
            params[:, 32 * c:32 * c + 32] = gW1[k, 128 * hf:128 * (hf + 1), :]
        params[rs, OFF_W2BD + 128 * g + 32 * j:OFF_W2BD + 128 * g + 32 * j + 32] = gW2[k]
        # dh1 weights with -gW3 folded in:
        # lhsT[32j+g', 32j+h] = -gW3[k][g'] * gW2[k][h, g']
        params[rs, OFF_W2TBD + 128 * g + 32 * j:OFF_W2TBD + 128 * g + 32 * j + 32] = \
            -gW3[k][:, None] * gW2[k].T
        biases[rs, g] = gb1[k]
        biases[rs, 3 + g] = gb2[k]
    # paired block-diagonal W1^T for the d_z matmuls (not negated: du1
    # already carries the sign flip from the folded -gW3)
    for m in range(4):
        base = 64 * (m % 2)
        for s in range(2):
            k = 2 * m + s
            params[base + 32 * s:base + 32 * s + 32,
                   OFF_W1T + 512 * m + 256 * s:OFF_W1T + 512 * m + 256 * (s + 1)] = \
                gW1[k].T
    params[0:32, OFF_W1T + 2048:OFF_W1T + 2048 + 256] = gW1[8].T

    cparams = np.zeros((128, CW), np.float32)
    for ih in range(2):
        for k in range(P_MAX):            # block k pairs with lag r = 8-k
            cparams[:, 2048 * ih + 256 * k:2048 * ih + 256 * (k + 1)] = \
                -W[7 - k][128 * ih:128 * (ih + 1), :]
    for r in range(1, P_MAX + 1):
        for jh in range(2):
            i = 2 * (r - 1) + jh
            cparams[:, OFF_WFUT + 256 * i:OFF_WFUT + 256 * (i + 1)] = \
                -W[r - 1].T[128 * jh:128 * (jh + 1), :]
    return {"params": params.astype(mdt), "cparams": cparams.astype(mdt),
            "biases": biases}


def run(inputs, trace=False):
    nc = _build()
    mdt = _DT[MM_MODE][1]
    params = _prep_params(
        np.asarray(inputs["gW1"]), np.asarray(inputs["gb1"]),
        np.asarray(inputs["gW2"]), np.asarray(inputs["gb2"]),
        np.asarray(inputs["gW3"]), np.asarray(inputs["gb3"]),
        np.asarray(inputs["W"]))
    z = np.asarray(inputs["z"])
    in_maps = []
    for ci in range(N_CORES):
        zc = z[ci * B_CORE:(ci + 1) * B_CORE]
        # zt[t, p, 512*c + b] = zc[512t + b, 128c + p]
        ztc = np.ascontiguousarray(
            zc.reshape(NBT, BT, 2 * NB, 128).transpose(0, 3, 2, 1)
        ).reshape(NBT, 128, ZW).astype(mdt)
        in_maps.append({"zt": ztc, **params})
    res = run_bass_kernel_spmd(nc, in_maps, core_ids=list(range(N_CORES)),
                               trace=trace)
    out = np.concatenate([np.asarray(r["out"], dtype=np.float32)
                          for r in res.results], axis=0)
    return out, res


def kernel(**inputs) -> np.ndarray:
    out, _ = run(inputs, trace=False)
    return out


# revision 30
# speedup vs baseline: 1.1762x; 1.1762x over previous
"""TRN2 Bass kernel for nn_MultiBlockStructuredScoreNet.

Computes s(z) = -grad_z U(z) where
  U(z) = sum_k MLP_k(z_k) + sum_r z_8^T W_r z_{8-r}
for z of shape (8192, 9*256), data-parallel over 8 NeuronCores.

Per core (1024 samples):
 - Host pre-transposes z to neuron-major (zT) so the PE contracts over the
   neuron dim with no on-chip transposes; host packs/transposes/sign-flips
   the small parameters so PSUM accumulation directly yields the score.
 - fp16 PE dtype and fp16 stores (cast back to f32 on host): rel err
   ~5e-4 against the 2e-2 gate, and the store traffic halves vs f32.
 - u1 lhsT tiles are densely packed (32 cols each) and the params DMA is
   split so the HAM warm-up matmuls only wait on a 147KB slice.
 - Emission order bridges DMA waits with real work: tile-1's MLP forward
   runs while the coupling matrices stream, so the PE never idles on the
   cparams DMA (the old kernel burned throwaway matmuls there).
 - MLP: u1 via col-tiled [K=128,M=32] matmuls packing 4 blocks per PSUM
   tile; u2/dh1 via block-diagonal [128,128] weights with -gW3 pre-folded
   into the dh1 weights; SiLU / SiLU' from the ACT table with fused bias,
   batched by function to avoid table reloads.
 - Cross couplings + MLP d_z accumulate b-major into PSUM per 128-sample
   chunk; adjacent blocks are paired into single N=512 matmuls (d_lag via
   adjacent wlag packing, d_z via block-diagonal W1^T pairs) to halve the
   weight-load count.  PSUM->SBUF copies split across DVE/ACT; stores are
   fp16 and the final chunk stores in 3 pieces to shorten the tail.
"""

import numpy as np
import ml_dtypes

import concourse.bass as bass
import concourse.tile as tile
from concourse import bacc, mybir
from concourse.bass_utils import run_bass_kernel_spmd

AF = mybir.ActivationFunctionType
F32 = mybir.dt.float32

N_CORES = 8
BATCH = 8192
B_CORE = BATCH // N_CORES     # 1024
BT = 512                      # batch tile (PSUM free-dim max for f32)
NBT = B_CORE // BT            # 2 batch tiles per core
NCHUNK = 4                    # 128-sample chunks per batch tile
NB = 9                        # blocks
P_MAX = 8
NN = 256                      # neurons per block
D = NB * NN                   # 2304
H = 32

ZW = 2 * NB * BT              # zt tile cols per batch tile: 18 chunks x 512
OFF_U1 = 0                    # dense u1 lhsT tiles: 18 x 32 cols
OFF_W1T = 18 * 32             # 576: paired block-diag W1^T region
OFF_W2BD = OFF_W1T + 4 * 512 + 256   # 2880
OFF_W2TBD = OFF_W2BD + 3 * 128       # 3264
PW = OFF_W2TBD + 3 * 128             # 3648
PA_SPLIT = OFF_W1T            # params DMA A/B split: warmup+u1 vs the rest
OFF_WFUT = 2 * P_MAX * NN     # 4096
CW = 2 * OFF_WFUT             # 8192

MM_MODE = "fp16"              # "fp16" | "bf16"

_DT = {
    "fp16": (mybir.dt.float16, np.float16),
    "bf16": (mybir.dt.bfloat16, ml_dtypes.bfloat16),
}


def _body(tc, out, zt, params, cparams, biases, ctx):
    nc = tc.nc
    sdt = _DT[MM_MODE][0]

    const = ctx.enter_context(tc.tile_pool(name="const", bufs=1))
    ztp = ctx.enter_context(tc.tile_pool(name="ztp", bufs=2))
    mlpp = ctx.enter_context(tc.tile_pool(name="mlpp", bufs=3, space="PSUM"))
    actp = ctx.enter_context(tc.tile_pool(name="actp", bufs=8))
    du1p = ctx.enter_context(tc.tile_pool(name="du1p", bufs=6))
    outp = ctx.enter_context(tc.tile_pool(name="outp", bufs=5, space="PSUM"))
    outs = ctx.enter_context(tc.tile_pool(name="outs", bufs=4))

    # ---- inputs -> SBUF in fine-grained pieces ordered by PE consumption:
    # u1 weights (slice A) + the first z chunks arrive ~7.5us in so the real
    # u1 matmuls double as the HAM ramp (no throwaway warm-ups); the HAM
    # clamps the PE hard if the ramp is followed by idle-then-burst, so the
    # pieces trickle in to keep the PE continuously fed from first matmul
    # to last.  cparams lands before zt1: the tile-0 cross chunks only need
    # cparams + zt0 (their dz matmuls come last, after the W1^T slice B2).
    pa_sb = const.tile([128, PW], sdt, name="pa")
    nc.sync.dma_start(pa_sb[:, 0:PA_SPLIT], params[:, 0:PA_SPLIT])
    bias_sb = const.tile([128, 6], F32, name="biassb")
    nc.sync.dma_start(bias_sb[:], biases[:])
    zt_sb = [ztp.tile([128, ZW], sdt, tag="zt", name="ztsb") for _ in range(NBT)]
    cp_sb = const.tile([128, CW], sdt, name="cp")
    nc.sync.dma_start(zt_sb[0][:, 16 * BT:ZW], zt[0, :, 16 * BT:ZW])
    for q in range(4):
        nc.sync.dma_start(zt_sb[0][:, 4 * BT * q:4 * BT * (q + 1)],
                          zt[0, :, 4 * BT * q:4 * BT * (q + 1)])
    nc.sync.dma_start(pa_sb[:, OFF_W2BD:PW], params[:, OFF_W2BD:PW])   # B1
    nc.sync.dma_start(zt_sb[1][:, 16 * BT:ZW], zt[1, :, 16 * BT:ZW])
    for q in range(4):
        nc.sync.dma_start(zt_sb[1][:, 4 * BT * q:4 * BT * (q + 1)],
                          zt[1, :, 4 * BT * q:4 * BT * (q + 1)])
    # cp in fine pieces so the cross phase starts as a ramp, not a cliff:
    # dlag bank p needs only its contiguous 1024-col piece (repacked on
    # host), dfut's pieces follow in consumption order, and the dz weights
    # (B2) interleave where the chunk needs them.  No arrival cliff ->
    # no idle-then-burst for the HAM to punish, whatever the DMA jitter.
    for q in range(4):
        nc.sync.dma_start(cp_sb[:, 1024 * q:1024 * (q + 1)],
                          cparams[:, 1024 * q:1024 * (q + 1)])
    nc.sync.dma_start(cp_sb[:, 4096:5120], cparams[:, 4096:5120])
    nc.sync.dma_start(cp_sb[:, 5120:6144], cparams[:, 5120:6144])
    nc.sync.dma_start(pa_sb[:, PA_SPLIT:PA_SPLIT + 1024],
                      params[:, PA_SPLIT:PA_SPLIT + 1024])               # B2a
    nc.sync.dma_start(cp_sb[:, 6144:7168], cparams[:, 6144:7168])
    nc.sync.dma_start(cp_sb[:, 7168:8192], cparams[:, 7168:8192])
    nc.sync.dma_start(pa_sb[:, PA_SPLIT + 1024:OFF_W2BD],
                      params[:, PA_SPLIT + 1024:OFF_W2BD])               # B2b

    def ztsl(t, c, c0, w):    # [128, w] slice at offset c0 of zT chunk c
        return zt_sb[t][:, BT * c + c0:BT * c + c0 + w]

    u1_state = {}
    act_state = {}
    du1_tiles = {}

    def u1_group(t, g):
        nblk = 4 if g < 2 else 1
        P = 32 * nblk
        u1 = mlpp.tile([128, BT], F32, tag="mlpp", name="u1t")
        for j in range(nblk):
            k = 4 * g + j
            for hf in range(2):
                c = 2 * k + hf
                nc.tensor.matmul(
                    u1[32 * j:32 * j + 32, :],
                    pa_sb[:, 32 * c:32 * c + 32],
                    ztsl(t, c, 0, BT), start=(hf == 0), stop=(hf == 1),
                    tile_position=(0, 32 * j))
        return u1, P

    def u1_phase(t, order=(0, 1, 2)):
        u1s = [None] * 3
        for g in order:
            u1s[g] = u1_group(t, g)
        u1_state[t] = u1s

    def mlp_act_phase(t):
        # SiLU + SiLU' on ACT only -- emitted early so the ACT engine can
        # run these under PE work.
        u1s = u1_state[t]
        h1s, sp1s = [], []
        for g in range(3):
            u1, P = u1s[g]
            h1 = actp.tile([128, BT], sdt, tag="act", name="h1t")
            nc.scalar.activation(h1[:P], u1[:P], AF.Silu, bias=bias_sb[:P, g:g + 1])
            h1s.append(h1)
        for g in range(3):
            u1, P = u1s[g]
            sp1 = actp.tile([128, BT], F32, tag="act", name="sp1t")
            nc.scalar.activation(sp1[:P], u1[:P], AF.Derivative_silu,
                                 bias=bias_sb[:P, g:g + 1])
            sp1s.append(sp1)
        act_state[t] = (h1s, sp1s)

    bwd_state = {}

    def mlp_u2_phase(t):
        # u2 matmuls + sp2: first half of the backward chain
        h1s, sp1s = act_state[t]
        u1s = u1_state[t]
        u2s, sp2s = [], []
        for g in range(3):
            P = u1s[g][1]
            u2 = mlpp.tile([128, BT], F32, tag="mlpp", name="u2t")
            nc.tensor.matmul(u2[:P], pa_sb[:P, OFF_W2BD + 128 * g:OFF_W2BD + 128 * g + P],
                             h1s[g][:P], start=True, stop=True)
            u2s.append(u2)
        for g in range(3):
            P = u1s[g][1]
            sp2 = actp.tile([128, BT], sdt, tag="act", name="sp2t")
            nc.scalar.activation(sp2[:P], u2s[g][:P], AF.Derivative_silu,
                                 bias=bias_sb[:P, 3 + g:4 + g])
            sp2s.append(sp2)
        bwd_state[t] = sp2s

    def mlp_bwd_phase(t):
        # dh1 matmuls + du1: second half of the backward chain
        h1s, sp1s = act_state[t]
        u1s = u1_state[t]
        sp2s = bwd_state[t]
        du1_sb = []
        for g in range(3):
            P = u1s[g][1]
            dh1 = mlpp.tile([128, BT], F32, tag="mlpp", name="dh1t")
            # -gW3 is folded into these weights: dh1 here is -d(e)/d(h1)
            nc.tensor.matmul(dh1[:P],
                             pa_sb[:P, OFF_W2TBD + 128 * g:OFF_W2TBD + 128 * g + P],
                             sp2s[g][:P], start=True, stop=True)
            du1 = du1p.tile([128, BT], sdt, tag="du1", name="du1t")
            nc.vector.tensor_mul(du1[:P], dh1[:P], sp1s[g][:P])
            du1_sb.append(du1)
        du1_tiles[t] = du1_sb

    def cross_chunk(t, c, dve_only=False, pipelined=False, last=False,
                    tiles=None):
        du1_sb = du1_tiles[t]
        bs = slice(128 * c, 128 * c + 128)
        ot = outs.tile([128, D], sdt, tag="outs", name="outst")
        if tiles is None:
            op = [outp.tile([128, 512], F32, tag="outp", name="outpt")
                  for _ in range(4)]
            o8 = outp.tile([128, 512], F32, tag="outp", name="outpt")
        else:
            op, o8 = tiles

        def dlag(p):
            for ih in range(2):
                nc.tensor.matmul(
                    op[p][:], ztsl(t, 2 * P_MAX + ih, 128 * c, 128),
                    cp_sb[:, 1024 * p + 512 * ih:1024 * p + 512 * ih + 512],
                    start=(ih == 0), stop=False)

        def dz(p):
            base = 64 * (p % 2)
            nc.tensor.matmul(
                op[p][:], du1_sb[p // 2][base:base + 64, bs],
                pa_sb[base:base + 64, OFF_W1T + 512 * p:OFF_W1T + 512 * p + 512],
                start=False, stop=True, tile_position=(base, 0))

        def copy(p, eng):
            if eng == 'v':
                nc.vector.tensor_copy(ot[:, 512 * p:512 * (p + 1)], op[p][:])
            else:
                nc.scalar.activation(ot[:, 512 * p:512 * (p + 1)], op[p][:], AF.Copy)

        def dfut():
            for r in range(1, P_MAX + 1):
                for jh in range(2):
                    i = 2 * (r - 1) + jh
                    nc.tensor.matmul(
                        o8[:, :NN], ztsl(t, 2 * (P_MAX - r) + jh, 128 * c, 128),
                        cp_sb[:, OFF_WFUT + 256 * i:OFF_WFUT + 256 * i + 256],
                        start=(i == 0), stop=False)
            nc.tensor.matmul(o8[:, :NN], du1_sb[2][0:32, bs],
                             pa_sb[0:32, OFF_W1T + 2048:OFF_W1T + 2048 + 256],
                             start=False, stop=True, tile_position=(0, 0))

        r0 = t * BT + c * 128
        if pipelined:
            # dfut first so the tiny [2048:2304] store leaves early; the
            # last chunk splits its p3 bank into 256-col halves so the
            # final compute->copy->store tail is only ~0.6us.
            dfut()
            nc.vector.tensor_copy(ot[:, 2048:2304], o8[:, :NN])
            nc.sync.dma_start(out[r0:r0 + 128, 2048:2304], ot[:, 2048:2304])
            for p in range(4):
                dlag(p)
                dz(p)
                if last and p == 3:
                    # split the final copy across ACT+DVE so it drains fast
                    nc.scalar.activation(ot[:, 1536:1792], op[3][:, 0:256],
                                         AF.Copy)
                    nc.vector.tensor_copy(ot[:, 1792:2048], op[3][:, 256:512])
                else:
                    copy(p, 'v' if p % 2 == 0 else 's')
                if p == 1:
                    nc.sync.dma_start(out[r0:r0 + 128, 0:1024], ot[:, 0:1024])
                if last and p == 2:
                    nc.sync.dma_start(out[r0:r0 + 128, 1024:1536],
                                      ot[:, 1024:1536])
            if last:
                nc.sync.dma_start(out[r0:r0 + 128, 1536:2048], ot[:, 1536:2048])
            else:
                nc.sync.dma_start(out[r0:r0 + 128, 1024:2048], ot[:, 1024:2048])
            return
        else:
            for p in range(4):
                dlag(p)
            dfut()
            for p in range(4):
                dz(p)
            for p in range(3):
                copy(p, 'v')
            if dve_only:
                copy(3, 'v')
                nc.vector.tensor_copy(ot[:, 2048:2304], o8[:, :NN])
            else:
                copy(3, 's')
                nc.scalar.activation(ot[:, 2048:2304], o8[:, :NN], AF.Copy)
        nc.sync.dma_start(out[r0:r0 + 128, :], ot[:])

    # ---- emission order == per-engine execution order.  Tile-0 MLP ramps
    # the HAM, tile-0 cross chunks run as cparams trickles in, tile-1 MLP
    # slots in once zt1 lands, then the tile-1 cross chunks.  Filler
    # matmuls (into chunk (0,0)'s pre-allocated PSUM banks) plug the
    # data-wait gaps so the PE stays continuously active: the HAM grants
    # full rate only after ~3us of unbroken activity, and punishes
    # idle-then-burst with a multi-us 50% clamp.
    op0 = [outp.tile([128, 512], F32, tag="outp", name="outpt")
           for _ in range(4)]
    o80 = outp.tile([128, 512], F32, tag="outp", name="outpt")

    def filler(n):
        for _ in range(n):
            nc.tensor.matmul(op0[0][:], pa_sb[:, 0:128], pa_sb[:, 0:BT],
                             start=True, stop=True)

    u1s0 = [None] * 3
    u1s0[2] = u1_group(0, 2)         # g2 first: its zT chunks arrive first
    filler(3)
    u1s0[0] = u1_group(0, 0)
    filler(4)
    u1s0[1] = u1_group(0, 1)
    u1_state[0] = u1s0
    mlp_act_phase(0)
    mlp_u2_phase(0)
    mlp_bwd_phase(0)
    u1_phase(1, order=(2, 0, 1))     # zt1 trickles in during this
    mlp_act_phase(1)
    filler(2)
    cross_chunk(0, 0, tiles=(op0, o80))
    cross_chunk(0, 1)
    mlp_u2_phase(1)                  # PE bits hide between cross chunks;
    cross_chunk(0, 2)                # the ACT round-trips run off-path
    mlp_bwd_phase(1)
    cross_chunk(0, 3)
    cross_chunk(1, 0, pipelined=True)
    cross_chunk(1, 1, pipelined=True)
    cross_chunk(1, 2, pipelined=True)
    cross_chunk(1, 3, pipelined=True, last=True)


# ------------------------------------------------------------- build + launch

_CACHED = {}


def _build():
    if MM_MODE in _CACHED:
        return _CACHED[MM_MODE]
    sdt = _DT[MM_MODE][0]
    nc = bacc.Bacc("TRN2", target_bir_lowering=False, debug=False,
                   num_devices=N_CORES)
    zt = nc.dram_tensor("zt", [NBT, 128, ZW], sdt, kind="ExternalInput").ap()
    params = nc.dram_tensor("params", [128, PW], sdt, kind="ExternalInput").ap()
    cparams = nc.dram_tensor("cparams", [128, CW], sdt, kind="ExternalInput").ap()
    biases = nc.dram_tensor("biases", [128, 6], F32, kind="ExternalInput").ap()
    out = nc.dram_tensor("out", [B_CORE, D], sdt, kind="ExternalOutput").ap()

    from contextlib import ExitStack
    with tile.TileContext(nc) as tc:
        with ExitStack() as ctx:
            _body(tc, out, zt, params, cparams, biases, ctx)
    nc.compile()
    _CACHED[MM_MODE] = nc
    return nc


def _prep_params(gW1, gb1, gW2, gb2, gW3, gb3, W):
    mdt = _DT[MM_MODE][1]
    params = np.zeros((128, PW), np.float32)
    biases = np.zeros((128, 6), np.float32)
    for k in range(NB):
        g, j = k // 4, k % 4
        rs = slice(32 * j, 32 * j + 32)
        for hf in range(2):
            # u1 lhsT tile (2k+hf), densely packed at 32 cols each
            c = 2 * k + hf
            params[:, 32 * c:32 * c + 32] = gW1[k, 128 * hf:128 * (hf + 1), :]
        params[rs, OFF_W2BD + 128 * g + 32 * j:OFF_W2BD + 128 * g + 32 * j + 32] = gW2[k]
        # dh1 weights with -gW3 folded in:
        # lhsT[32j+g', 32j+h] = -gW3[k][g'] * gW2[k][h, g']
        params[rs, OFF_W2TBD + 128 * g + 32 * j:OFF_W2TBD + 128 * g + 32 * j + 32] = \
            -gW3[k][:, None] * gW2[k].T
        biases[rs, g] = gb1[k]
        biases[rs, 3 + g] = gb2[k]
    # paired block-diagonal W1^T for the d_z matmuls (not negated: du1
    # already carries the sign flip from the folded -gW3)
    for m in range(4):
        base = 64 * (m % 2)
        for s in range(2):
            k = 2 * m + s
            params[base + 32 * s:base + 32 * s + 32,
                   OFF_W1T + 512 * m + 256 * s:OFF_W1T + 512 * m + 256 * (s + 1)] = \
                gW1[k].T
    params[0:32, OFF_W1T + 2048:OFF_W1T + 2048 + 256] = gW1[8].T

    cparams = np.zeros((128, CW), np.float32)
    # dlag region repacked bank-major: bank p's piece [1024p:1024(p+1)]
    # holds both k-halves of blocks 2p, 2p+1 (block k pairs with lag 8-k)
    for p in range(4):
        for ih in range(2):
            for s in range(2):
                k = 2 * p + s
                cparams[:, 1024 * p + 512 * ih + 256 * s:
                        1024 * p + 512 * ih + 256 * (s + 1)] = \
                    -W[7 - k][128 * ih:128 * (ih + 1), :]
    for r in range(1, P_MAX + 1):
        for jh in range(2):
            i = 2 * (r - 1) + jh
            cparams[:, OFF_WFUT + 256 * i:OFF_WFUT + 256 * (i + 1)] = \
                -W[r - 1].T[128 * jh:128 * (jh + 1), :]
    return {"params": params.astype(mdt), "cparams": cparams.astype(mdt),
            "biases": biases}


def run(inputs, trace=False):
    nc = _build()
    mdt = _DT[MM_MODE][1]
    params = _prep_params(
        np.asarray(inputs["gW1"]), np.asarray(inputs["gb1"]),
        np.asarray(inputs["gW2"]), np.asarray(inputs["gb2"]),
        np.asarray(inputs["gW3"]), np.asarray(inputs["gb3"]),
        np.asarray(inputs["W"]))
    z = np.asarray(inputs["z"])
    in_maps = []
    for ci in range(N_CORES):
        zc = z[ci * B_CORE:(ci + 1) * B_CORE]
        # zt[t, p, 512*c + b] = zc[512t + b, 128c + p]
        ztc = np.ascontiguousarray(
            zc.reshape(NBT, BT, 2 * NB, 128).transpose(0, 3, 2, 1)
        ).reshape(NBT, 128, ZW).astype(mdt)
        in_maps.append({"zt": ztc, **params})
    res = run_bass_kernel_spmd(nc, in_maps, core_ids=list(range(N_CORES)),
                               trace=trace)
    out = np.concatenate([np.asarray(r["out"], dtype=np.float32)
                          for r in res.results], axis=0)
    return out, res


def kernel(**inputs) -> np.ndarray:
    out, _ = run(inputs, trace=False)
    return out


# revision 37
# speedup vs baseline: 1.1952x; 1.0161x over previous
"""TRN2 Bass kernel for nn_MultiBlockStructuredScoreNet.

Computes s(z) = -grad_z U(z) where
  U(z) = sum_k MLP_k(z_k) + sum_r z_8^T W_r z_{8-r}
for z of shape (8192, 9*256), data-parallel over 8 NeuronCores.

Per core (1024 samples):
 - Host pre-transposes z to neuron-major (zT) so the PE contracts over the
   neuron dim with no on-chip transposes; host packs/transposes/sign-flips
   the small parameters so PSUM accumulation directly yields the score.
 - fp16 PE dtype and fp16 stores (cast back to f32 on host): rel err
   ~5e-4 against the 2e-2 gate, and the store traffic halves vs f32.
 - u1 lhsT tiles are densely packed (32 cols each) and the params DMA is
   split so the HAM warm-up matmuls only wait on a 147KB slice.
 - Emission order bridges DMA waits with real work: tile-1's MLP forward
   runs while the coupling matrices stream, so the PE never idles on the
   cparams DMA (the old kernel burned throwaway matmuls there).
 - MLP: u1 via col-tiled [K=128,M=32] matmuls packing 4 blocks per PSUM
   tile; u2/dh1 via block-diagonal [128,128] weights with -gW3 pre-folded
   into the dh1 weights; SiLU / SiLU' from the ACT table with fused bias,
   batched by function to avoid table reloads.
 - Cross couplings + MLP d_z accumulate b-major into PSUM per 128-sample
   chunk; adjacent blocks are paired into single N=512 matmuls (d_lag via
   adjacent wlag packing, d_z via block-diagonal W1^T pairs) to halve the
   weight-load count.  PSUM->SBUF copies split across DVE/ACT; stores are
   fp16 and the final chunk stores in 3 pieces to shorten the tail.
"""

import numpy as np
import ml_dtypes

import concourse.bass as bass
import concourse.tile as tile
from concourse import bacc, mybir
from concourse.bass_utils import run_bass_kernel_spmd

AF = mybir.ActivationFunctionType
F32 = mybir.dt.float32

N_CORES = 8
BATCH = 8192
B_CORE = BATCH // N_CORES     # 1024
BT = 512                      # batch tile (PSUM free-dim max for f32)
NBT = B_CORE // BT            # 2 batch tiles per core
NCHUNK = 4                    # 128-sample chunks per batch tile
NB = 9                        # blocks
P_MAX = 8
NN = 256                      # neurons per block
D = NB * NN                   # 2304
H = 32

ZW = 2 * NB * BT              # zt tile cols per batch tile: 18 chunks x 512
OFF_U1 = 0                    # dense u1 lhsT tiles: 18 x 32 cols
OFF_BIAS = 18 * 32            # 576: 6 cols of fp16 ACT biases ride slice A
PA_SPLIT = OFF_BIAS + 6       # 582: params DMA A/B split
OFF_W1T = PA_SPLIT            # paired block-diag W1^T region (B2)
OFF_W2BD = OFF_W1T + 4 * 512 + 256   # 2886
OFF_W2TBD = OFF_W2BD + 3 * 128       # 3270
PW = OFF_W2TBD + 3 * 128             # 3654
OFF_WFUT = 2 * P_MAX * NN     # 4096
CW = 2 * OFF_WFUT             # 8192

MM_MODE = "fp16"              # "fp16" | "bf16"

_DT = {
    "fp16": (mybir.dt.float16, np.float16),
    "bf16": (mybir.dt.bfloat16, ml_dtypes.bfloat16),
}


def _body(tc, out, zt, params, cparams, ctx):
    nc = tc.nc
    sdt = _DT[MM_MODE][0]

    const = ctx.enter_context(tc.tile_pool(name="const", bufs=1))
    ztp = ctx.enter_context(tc.tile_pool(name="ztp", bufs=2))
    mlpp = ctx.enter_context(tc.tile_pool(name="mlpp", bufs=3, space="PSUM"))
    actp = ctx.enter_context(tc.tile_pool(name="actp", bufs=8))
    du1p = ctx.enter_context(tc.tile_pool(name="du1p", bufs=6))
    outp = ctx.enter_context(tc.tile_pool(name="outp", bufs=5, space="PSUM"))
    outs = ctx.enter_context(tc.tile_pool(name="outs", bufs=4))

    # ---- inputs -> SBUF in fine-grained pieces ordered by PE consumption:
    # u1 weights (slice A) + the first z chunks arrive ~7.5us in so the real
    # u1 matmuls double as the HAM ramp (no throwaway warm-ups); the HAM
    # clamps the PE hard if the ramp is followed by idle-then-burst, so the
    # pieces trickle in to keep the PE continuously fed from first matmul
    # to last.  cparams lands before zt1: the tile-0 cross chunks only need
    # cparams + zt0 (their dz matmuls come last, after the W1^T slice B2).
    # Every dma_start costs ~0.6us of serial trigger-issue on the Sync
    # engine, and the trigger sem pool recycles every ~8 transfers (trigger
    # N waits transfer N-8 completion), so the input stream is exactly 12
    # triggers: big pieces, ordered by PE consumption.
    pa_sb = const.tile([128, PW], sdt, name="pa")
    nc.sync.dma_start(pa_sb[:, 0:PA_SPLIT], params[:, 0:PA_SPLIT])     # A
    bias_sb = pa_sb[:, OFF_BIAS:OFF_BIAS + 6]
    zt_sb = [ztp.tile([128, ZW], sdt, tag="zt", name="ztsb") for _ in range(NBT)]
    cp_sb = const.tile([128, CW], sdt, name="cp")
    nc.sync.dma_start(zt_sb[0][:, 16 * BT:ZW], zt[0, :, 16 * BT:ZW])
    for a, b in [(0, 6 * BT), (6 * BT, 12 * BT), (12 * BT, 16 * BT)]:
        nc.sync.dma_start(zt_sb[0][:, a:b], zt[0, :, a:b])
    nc.sync.dma_start(pa_sb[:, OFF_W2BD:PW], params[:, OFF_W2BD:PW])   # B1
    nc.sync.dma_start(zt_sb[1][:, 16 * BT:ZW], zt[1, :, 16 * BT:ZW])
    for a, b in [(0, 8 * BT), (8 * BT, 16 * BT)]:
        nc.sync.dma_start(zt_sb[1][:, a:b], zt[1, :, a:b])
    nc.sync.dma_start(cp_sb[:, 0:4096], cparams[:, 0:4096])            # dlag
    nc.sync.dma_start(cp_sb[:, 4096:8192], cparams[:, 4096:8192])      # dfut
    nc.sync.dma_start(pa_sb[:, PA_SPLIT:OFF_W2BD], params[:, PA_SPLIT:OFF_W2BD])  # B2

    def ztsl(t, c, c0, w):    # [128, w] slice at offset c0 of zT chunk c
        return zt_sb[t][:, BT * c + c0:BT * c + c0 + w]

    u1_state = {}
    act_state = {}
    du1_tiles = {}

    def u1_group(t, g):
        nblk = 4 if g < 2 else 1
        P = 32 * nblk
        u1 = mlpp.tile([128, BT], F32, tag="mlpp", name="u1t")
        for j in range(nblk):
            k = 4 * g + j
            for hf in range(2):
                c = 2 * k + hf
                nc.tensor.matmul(
                    u1[32 * j:32 * j + 32, :],
                    pa_sb[:, 32 * c:32 * c + 32],
                    ztsl(t, c, 0, BT), start=(hf == 0), stop=(hf == 1),
                    tile_position=(0, 32 * j))
        return u1, P

    def u1_phase(t, order=(0, 1, 2)):
        u1s = [None] * 3
        for g in order:
            u1s[g] = u1_group(t, g)
        u1_state[t] = u1s

    def mlp_act_phase(t):
        # SiLU + SiLU' on ACT only -- emitted early so the ACT engine can
        # run these under PE work.
        u1s = u1_state[t]
        h1s, sp1s = [], []
        for g in range(3):
            u1, P = u1s[g]
            h1 = actp.tile([128, BT], sdt, tag="act", name="h1t")
            nc.scalar.activation(h1[:P], u1[:P], AF.Silu, bias=bias_sb[:P, g:g + 1])
            h1s.append(h1)
        for g in range(3):
            u1, P = u1s[g]
            sp1 = actp.tile([128, BT], F32, tag="act", name="sp1t")
            nc.scalar.activation(sp1[:P], u1[:P], AF.Derivative_silu,
                                 bias=bias_sb[:P, g:g + 1])
            sp1s.append(sp1)
        act_state[t] = (h1s, sp1s)

    bwd_state = {}

    def mlp_u2_phase(t):
        # u2 matmuls + sp2: first half of the backward chain
        h1s, sp1s = act_state[t]
        u1s = u1_state[t]
        u2s, sp2s = [], []
        for g in range(3):
            P = u1s[g][1]
            u2 = mlpp.tile([128, BT], F32, tag="mlpp", name="u2t")
            nc.tensor.matmul(u2[:P], pa_sb[:P, OFF_W2BD + 128 * g:OFF_W2BD + 128 * g + P],
                             h1s[g][:P], start=True, stop=True)
            u2s.append(u2)
        for g in range(3):
            P = u1s[g][1]
            sp2 = actp.tile([128, BT], sdt, tag="act", name="sp2t")
            nc.scalar.activation(sp2[:P], u2s[g][:P], AF.Derivative_silu,
                                 bias=bias_sb[:P, 3 + g:4 + g])
            sp2s.append(sp2)
        bwd_state[t] = sp2s

    def mlp_bwd_phase(t):
        # dh1 matmuls + du1: second half of the backward chain
        h1s, sp1s = act_state[t]
        u1s = u1_state[t]
        sp2s = bwd_state[t]
        du1_sb = []
        for g in range(3):
            P = u1s[g][1]
            dh1 = mlpp.tile([128, BT], F32, tag="mlpp", name="dh1t")
            # -gW3 is folded into these weights: dh1 here is -d(e)/d(h1)
            nc.tensor.matmul(dh1[:P],
                             pa_sb[:P, OFF_W2TBD + 128 * g:OFF_W2TBD + 128 * g + P],
                             sp2s[g][:P], start=True, stop=True)
            du1 = du1p.tile([128, BT], sdt, tag="du1", name="du1t")
            nc.vector.tensor_mul(du1[:P], dh1[:P], sp1s[g][:P])
            du1_sb.append(du1)
        du1_tiles[t] = du1_sb

    def cross_chunk(t, c, dve_only=False, pipelined=False, last=False,
                    tiles=None):
        du1_sb = du1_tiles[t]
        bs = slice(128 * c, 128 * c + 128)
        ot = outs.tile([128, D], sdt, tag="outs", name="outst")
        if tiles is None:
            op = [outp.tile([128, 512], F32, tag="outp", name="outpt")
                  for _ in range(4)]
            o8 = outp.tile([128, 512], F32, tag="outp", name="outpt")
        else:
            op, o8 = tiles

        def dlag(p):
            for ih in range(2):
                nc.tensor.matmul(
                    op[p][:], ztsl(t, 2 * P_MAX + ih, 128 * c, 128),
                    cp_sb[:, 1024 * p + 512 * ih:1024 * p + 512 * ih + 512],
                    start=(ih == 0), stop=False)

        def dz(p):
            base = 64 * (p % 2)
            nc.tensor.matmul(
                op[p][:], du1_sb[p // 2][base:base + 64, bs],
                pa_sb[base:base + 64, OFF_W1T + 512 * p:OFF_W1T + 512 * p + 512],
                start=False, stop=True, tile_position=(base, 0))

        def copy(p, eng):
            if eng == 'v':
                nc.vector.tensor_copy(ot[:, 512 * p:512 * (p + 1)], op[p][:])
            else:
                nc.scalar.activation(ot[:, 512 * p:512 * (p + 1)], op[p][:], AF.Copy)

        def dfut():
            for r in range(1, P_MAX + 1):
                for jh in range(2):
                    i = 2 * (r - 1) + jh
                    nc.tensor.matmul(
                        o8[:, :NN], ztsl(t, 2 * (P_MAX - r) + jh, 128 * c, 128),
                        cp_sb[:, OFF_WFUT + 256 * i:OFF_WFUT + 256 * i + 256],
                        start=(i == 0), stop=False)
            nc.tensor.matmul(o8[:, :NN], du1_sb[2][0:32, bs],
                             pa_sb[0:32, OFF_W1T + 2048:OFF_W1T + 2048 + 256],
                             start=False, stop=True, tile_position=(0, 0))

        r0 = t * BT + c * 128
        if pipelined:
            # dfut first so the tiny [2048:2304] store leaves early; the
            # last chunk splits its p3 bank into 256-col halves so the
            # final compute->copy->store tail is only ~0.6us.
            dfut()
            nc.vector.tensor_copy(ot[:, 2048:2304], o8[:, :NN])
            nc.sync.dma_start(out[r0:r0 + 128, 2048:2304], ot[:, 2048:2304])
            for p in range(4):
                dlag(p)
                dz(p)
                if last and p == 3:
                    # split the final copy across ACT+DVE so it drains fast
                    nc.scalar.activation(ot[:, 1536:1792], op[3][:, 0:256],
                                         AF.Copy)
                    nc.vector.tensor_copy(ot[:, 1792:2048], op[3][:, 256:512])
                else:
                    copy(p, 'v' if p % 2 == 0 else 's')
                if p == 1:
                    nc.sync.dma_start(out[r0:r0 + 128, 0:1024], ot[:, 0:1024])
                if last and p == 2:
                    nc.sync.dma_start(out[r0:r0 + 128, 1024:1536],
                                      ot[:, 1024:1536])
            if last:
                nc.sync.dma_start(out[r0:r0 + 128, 1536:2048], ot[:, 1536:2048])
            else:
                nc.sync.dma_start(out[r0:r0 + 128, 1024:2048], ot[:, 1024:2048])
            return
        else:
            for p in range(4):
                dlag(p)
            dfut()
            for p in range(4):
                dz(p)
            for p in range(3):
                copy(p, 'v')
            if dve_only:
                copy(3, 'v')
                nc.vector.tensor_copy(ot[:, 2048:2304], o8[:, :NN])
            else:
                copy(3, 's')
                nc.scalar.activation(ot[:, 2048:2304], o8[:, :NN], AF.Copy)
        nc.sync.dma_start(out[r0:r0 + 128, :], ot[:])

    # ---- emission order == per-engine execution order.  Tile-0 MLP ramps
    # the HAM, tile-0 cross chunks run as cparams trickles in, tile-1 MLP
    # slots in once zt1 lands, then the tile-1 cross chunks.  Filler
    # matmuls (into chunk (0,0)'s pre-allocated PSUM banks) plug the
    # data-wait gaps so the PE stays continuously active: the HAM grants
    # full rate only after ~3us of unbroken activity, and punishes
    # idle-then-burst with a multi-us 50% clamp.
    op0 = [outp.tile([128, 512], F32, tag="outp", name="outpt")
           for _ in range(4)]
    o80 = outp.tile([128, 512], F32, tag="outp", name="outpt")

    def filler(n):
        for _ in range(n):
            nc.tensor.matmul(op0[0][:], pa_sb[:, 0:128], pa_sb[:, 0:BT],
                             start=True, stop=True)

    u1s0 = [None] * 3
    u1s0[2] = u1_group(0, 2)         # g2 first: its zT chunks arrive first
    filler(3)
    u1s0[0] = u1_group(0, 0)
    filler(2)
    u1s0[1] = u1_group(0, 1)
    u1_state[0] = u1s0
    mlp_act_phase(0)
    mlp_u2_phase(0)
    mlp_bwd_phase(0)
    u1_phase(1, order=(2, 0, 1))     # zt1 trickles in during this
    mlp_act_phase(1)
    filler(6)
    cross_chunk(0, 0, tiles=(op0, o80))
    cross_chunk(0, 1)
    mlp_u2_phase(1)                  # PE bits hide between cross chunks;
    cross_chunk(0, 2)                # the ACT round-trips run off-path
    mlp_bwd_phase(1)
    cross_chunk(0, 3)
    cross_chunk(1, 0, pipelined=True)
    cross_chunk(1, 1, pipelined=True)
    cross_chunk(1, 2, pipelined=True)
    cross_chunk(1, 3, pipelined=True, last=True)


# ------------------------------------------------------------- build + launch

_CACHED = {}


def _build():
    if MM_MODE in _CACHED:
        return _CACHED[MM_MODE]
    sdt = _DT[MM_MODE][0]
    nc = bacc.Bacc("TRN2", target_bir_lowering=False, debug=False,
                   num_devices=N_CORES)
    zt = nc.dram_tensor("zt", [NBT, 128, ZW], sdt, kind="ExternalInput").ap()
    params = nc.dram_tensor("params", [128, PW], sdt, kind="ExternalInput").ap()
    cparams = nc.dram_tensor("cparams", [128, CW], sdt, kind="ExternalInput").ap()
    out = nc.dram_tensor("out", [B_CORE, D], sdt, kind="ExternalOutput").ap()

    from contextlib import ExitStack
    with tile.TileContext(nc) as tc:
        with ExitStack() as ctx:
            _body(tc, out, zt, params, cparams, ctx)
    nc.compile()
    _CACHED[MM_MODE] = nc
    return nc


def _prep_params(gW1, gb1, gW2, gb2, gW3, gb3, W):
    mdt = _DT[MM_MODE][1]
    params = np.zeros((128, PW), np.float32)
    biases = params[:, OFF_BIAS:OFF_BIAS + 6]   # fp16 bias cols ride slice A
    for k in range(NB):
        g, j = k // 4, k % 4
        rs = slice(32 * j, 32 * j + 32)
        for hf in range(2):
            # u1 lhsT tile (2k+hf), densely packed at 32 cols each
            c = 2 * k + hf
            params[:, 32 * c:32 * c + 32] = gW1[k, 128 * hf:128 * (hf + 1), :]
        params[rs, OFF_W2BD + 128 * g + 32 * j:OFF_W2BD + 128 * g + 32 * j + 32] = gW2[k]
        # dh1 weights with -gW3 folded in:
        # lhsT[32j+g', 32j+h] = -gW3[k][g'] * gW2[k][h, g']
        params[rs, OFF_W2TBD + 128 * g + 32 * j:OFF_W2TBD + 128 * g + 32 * j + 32] = \
            -gW3[k][:, None] * gW2[k].T
        biases[rs, g] = gb1[k]
        biases[rs, 3 + g] = gb2[k]
    # paired block-diagonal W1^T for the d_z matmuls (not negated: du1
    # already carries the sign flip from the folded -gW3)
    for m in range(4):
        base = 64 * (m % 2)
        for s in range(2):
            k = 2 * m + s
            params[base + 32 * s:base + 32 * s + 32,
                   OFF_W1T + 512 * m + 256 * s:OFF_W1T + 512 * m + 256 * (s + 1)] = \
                gW1[k].T
    params[0:32, OFF_W1T + 2048:OFF_W1T + 2048 + 256] = gW1[8].T

    cparams = np.zeros((128, CW), np.float32)
    # dlag region repacked bank-major: bank p's piece [1024p:1024(p+1)]
    # holds both k-halves of blocks 2p, 2p+1 (block k pairs with lag 8-k)
    for p in range(4):
        for ih in range(2):
            for s in range(2):
                k = 2 * p + s
                cparams[:, 1024 * p + 512 * ih + 256 * s:
                        1024 * p + 512 * ih + 256 * (s + 1)] = \
                    -W[7 - k][128 * ih:128 * (ih + 1), :]
    for r in range(1, P_MAX + 1):
        for jh in range(2):
            i = 2 * (r - 1) + jh
            cparams[:, OFF_WFUT + 256 * i:OFF_WFUT + 256 * (i + 1)] = \
                -W[r - 1].T[128 * jh:128 * (jh + 1), :]
    return {"params": params.astype(mdt), "cparams": cparams.astype(mdt)}


def run(inputs, trace=False):
    nc = _build()
    mdt = _DT[MM_MODE][1]
    params = _prep_params(
        np.asarray(inputs["gW1"]), np.asarray(inputs["gb1"]),
        np.asarray(inputs["gW2"]), np.asarray(inputs["gb2"]),
        np.asarray(inputs["gW3"]), np.asarray(inputs["gb3"]),
        np.asarray(inputs["W"]))
    z = np.asarray(inputs["z"])
    in_maps = []
    for ci in range(N_CORES):
        zc = z[ci * B_CORE:(ci + 1) * B_CORE]
        # zt[t, p, 512*c + b] = zc[512t + b, 128c + p]
        ztc = np.ascontiguousarray(
            zc.reshape(NBT, BT, 2 * NB, 128).transpose(0, 3, 2, 1)
        ).reshape(NBT, 128, ZW).astype(mdt)
        in_maps.append({"zt": ztc, **params})
    res = run_bass_kernel_spmd(nc, in_maps, core_ids=list(range(N_CORES)),
                               trace=trace)
    out = np.concatenate([np.asarray(r["out"], dtype=np.float32)
                          for r in res.results], axis=0)
    return out, res


def kernel(**inputs) -> np.ndarray:
    out, _ = run(inputs, trace=False)
    return out
